# revision 10
# baseline (speedup 1.0000x reference)
# Bass/Trainium2 kernel for nn_Bilinear_46660524703902.
#
# Math (see reference):
#   s    = sum_n x2[n, :]                        # [R] global row-sum
#   M    = einsum('olr,r->lo', U, s)             # [L, O]
#   out  = x1 @ (M + W_l) + x2 @ W_r + N * bias  # [N, O]
#
# Distribution: data-parallel over the flattened row axis across 8 cores.
# M is linear in s, so each core computes M_c from its local row-sum and one
# 64KB AllReduce of M_c yields the global M.
#
# Dataflow (per core, rows=65536, 32 chunks of 16 128-row tiles):
#   Phase A: SWDGE cast-load x2 chunks fp32->bf16 on the gpsimd queue
#            (8KB/partition descriptors), transpose each 128x128 tile on the
#            PE (is_transpose matmul -> bf16 PSUM), ACT copies PSUM->SBUF
#            into the resident x2T [r, n] (16MB bf16), DVE row-sums.
#   M:       stream U' [r,(o l)] bf16 from DRAM through the ld2 pool in 8
#            pieces; 16 PE matmuls per piece (lhsT=U' slice, rhs=s) -> M_c;
#            AllReduce(M_c) on gpsimd; readback also on gpsimd.
#   Phase B: x1 loads run as plain fp32 HWDGE DMAs on the *sync* queue so
#            they keep streaming while gpsimd sits in the collective's
#            completion wait; DVE casts fp32->bf16 (it has slack), PE
#            transposes the bf16 tiles (tpsum+ACT copy like phase A), then
#            per 128-row tile two accumulating matmuls into PSUM [n, o]:
#              psum  = x2T_tile.T @ W_r
#              psum += x1T_tile.T @ A          (A = M + W_l)
#            DVE adds the pre-tiled N*bias writing bf16, stores bf16 to HBM
#            (4KB/partition descriptors), also on sync (fire-and-forget).
# No xbar DMA transposes and bf16 stores keep per-SDMA-engine work at
# ~260us vs the 2.16ms baseline's ~500us.

import numpy as np
import ml_dtypes
from contextlib import ExitStack

N_CORES = 8
FEAT = 128  # L == R == O == 128
CL = 16     # 128-row tiles per load chunk (2048 rows)
C = 8       # tiles per PSUM accumulation group

_nc_cache: dict = {}


def _build(rows_per_core: int, xbark: int = 0, obf32: bool = False):
    """Build + compile the per-core Bass module (same program on all cores).

    xbark: number of x1 tiles per chunk transposed via xbar DMA instead of
           the PE (0..CL), for PE/DMA load balancing.
    obf32: store fp32 output via SWDGE instead of bf16 via sync (fallback).
    """
    from concourse import bacc, mybir, tile

    f32 = mybir.dt.float32
    bf16 = mybir.dt.bfloat16
    fp8 = mybir.dt.float8e4
    X = mybir.AxisListType.X

    P = 128
    assert rows_per_core % (P * CL) == 0
    nlc = rows_per_core // (P * CL)  # load chunks per tensor (32 full size)
    odt = f32 if obf32 else bf16

    nc = bacc.Bacc("TRN2", target_bir_lowering=False, debug=False,
                   num_devices=N_CORES)

    x1 = nc.dram_tensor("input_left", [rows_per_core, FEAT], f32,
                        kind="ExternalInput")
    x2 = nc.dram_tensor("input_right", [rows_per_core, FEAT], f32,
                        kind="ExternalInput")
    up = nc.dram_tensor("u_prep", [FEAT, FEAT * FEAT], bf16,
                        kind="ExternalInput")  # [r, (o l)] = U[o, l, r]
    wl = nc.dram_tensor("w_l", [FEAT, FEAT], f32, kind="ExternalInput")
    wr = nc.dram_tensor("w_r", [FEAT, FEAT], fp8, kind="ExternalInput")
    biasT = nc.dram_tensor("bias_tiled", [P, C * FEAT], f32,
                           kind="ExternalInput")  # N*bias tiled C times
    idn = nc.dram_tensor("ident", [P, FEAT], bf16, kind="ExternalInput")
    out = nc.dram_tensor("out", [rows_per_core, FEAT], odt,
                         kind="ExternalOutput")

    # Row->partition mapping: within chunk j, partition p holds the 16
    # consecutive rows j*2048 + p*16 + k, so every HBM load runs 8KB
    # contiguous per partition (stores: 4KB bf16). Transposing tile k of a
    # chunk yields [r, p] columns where column p is row p*16+k, so psum
    # partition i of the phase-B matmuls corresponds to row i*16+k,
    # matching the (k o) store layout below.
    x2v = x2.ap().rearrange("(h p k) r -> h p k r", p=P, k=CL)
    x1v = x1.ap().rearrange("(h p k) r -> h p k r", p=P, k=CL)
    outv = out.ap().rearrange("(h p k) o -> h p (k o)", p=P, k=CL)
    upv = up.ap().rearrange("r (g f) -> g r f", g=8)

    with tile.TileContext(nc) as tc, ExitStack() as ctx:
        consts = ctx.enter_context(tc.tile_pool(name="consts", bufs=1))
        big = ctx.enter_context(tc.tile_pool(name="big", bufs=1))
        ld2 = ctx.enter_context(tc.tile_pool(name="ld2", bufs=3))
        ld1f = ctx.enter_context(tc.tile_pool(name="ld1f", bufs=4))
        ld1b = ctx.enter_context(tc.tile_pool(name="ld1b", bufs=6))
        x1tp = ctx.enter_context(tc.tile_pool(name="x1t", bufs=6))
        outp = ctx.enter_context(tc.tile_pool(name="outp", bufs=3))
        psum = ctx.enter_context(tc.tile_pool(name="psum", bufs=2, space="PSUM"))
        tpsum = ctx.enter_context(tc.tile_pool(name="tpsum", bufs=2, space="PSUM"))
        mpsum = ctx.enter_context(tc.tile_pool(name="mpsum", bufs=1, space="PSUM"))
        dram = ctx.enter_context(tc.tile_pool(name="dram", bufs=1, space="DRAM"))

        # Constants, loaded once
        wl_sb = consts.tile([FEAT, FEAT], f32)
        wr_sb = consts.tile([FEAT, FEAT], fp8)
        bias_sb = consts.tile([P, C * FEAT], f32)
        idn_sb = consts.tile([P, FEAT], bf16)
        nc.sync.dma_start(wl_sb[:], wl[:])
        nc.sync.dma_start(wr_sb[:], wr[:])
        nc.sync.dma_start(bias_sb[:], biasT[:])
        nc.sync.dma_start(idn_sb[:], idn[:])

        # Persistent working tiles
        x2t_all = big.tile([P, rows_per_core], fp8)  # [r, n] resident 8MB
        s_cols = consts.tile([P, max(4 * nlc, 1)], f32)
        s_f32 = consts.tile([P, 1], f32)
        s_bf = consts.tile([P, 1], bf16)
        m_sb = consts.tile([FEAT, FEAT], f32)
        mg_sb = consts.tile([FEAT, FEAT], f32)
        a_f32 = consts.tile([FEAT, FEAT], f32)
        a_bf = consts.tile([FEAT, FEAT], bf16)

        # ---- Phase A: stream x2, PE-transpose into residency, row-sums
        for j in range(nlc):
            x2n = ld2.tile([P, CL, FEAT], bf16)
            nc.gpsimd.dma_start(x2n[:], x2v[j])  # fp32 -> bf16 cast
            for g in range(CL // 4):
                tp = tpsum.tile([P, 4 * FEAT], bf16)
                for kk in range(4):
                    k = g * 4 + kk
                    nc.tensor.transpose(tp[:, kk * FEAT:(kk + 1) * FEAT],
                                        x2n[:, k, :], idn_sb[:])
                nc.scalar.copy(
                    x2t_all[:, (j * CL + g * 4) * P:(j * CL + g * 4 + 4) * P],
                    tp[:])
                nc.vector.reduce_sum(s_cols[:, j * 4 + g:j * 4 + g + 1],
                                     tp[:], axis=X)

        # ---- x1 loads: plain fp32 on the sync queue (HWDGE) so nothing
        # blocks them; DVE casts to bf16 into ld1b.
        x1n_tiles = [None] * nlc

        def emit_x1_load(j):
            x1f = ld1f.tile([P, CL, FEAT], f32)
            nc.sync.dma_start(x1f[:], x1v[j])
            x1b = ld1b.tile([P, CL, FEAT], bf16)
            nc.vector.tensor_copy(x1b[:], x1f[:])
            x1n_tiles[j] = x1b

        # ---- M_c = einsum(U, s_local): stream U' through ld2, 128 matmuls
        nc.vector.reduce_sum(s_f32[:], s_cols[:], axis=X)
        nc.vector.tensor_copy(s_bf[:], s_f32[:])
        m_ps = mpsum.tile([FEAT, FEAT], f32)
        for g in range(8):
            upg = ld2.tile([P, 16 * FEAT], bf16)
            nc.sync.dma_start(upg[:], upv[g])
            for oo in range(16):
                o = g * 16 + oo
                nc.tensor.matmul(m_ps[:, o:o + 1],
                                 upg[:, oo * FEAT:(oo + 1) * FEAT],
                                 s_bf[:], start=True, stop=True)
        nc.vector.tensor_copy(m_sb[:], m_ps[:])
        m_loc = dram.tile([FEAT, FEAT], f32)
        m_glob = dram.tile([FEAT, FEAT], f32)
        nc.sync.dma_start(m_loc[:], m_sb[:])

        # gpsimd queue: just the collective + readback; x1 loads and stores
        # never queue behind it.
        nc.gpsimd.collective_compute(
            "AllReduce", mybir.AluOpType.add,
            replica_groups=[list(range(N_CORES))],
            ins=[m_loc.opt()], outs=[m_glob.opt()])
        nc.gpsimd.dma_start(mg_sb[:], m_glob[:])

        # ---- Phase B: PE-transpose x1, matmuls, bias, store. Transposes
        # lead matmul emission by D chunks (= x1t bufs) so the PE queue
        # isn't head-blocked on the a_bf wait during the collective.
        x1t_tiles = [None] * nlc

        def emit_transposes(j):
            x1n = x1n_tiles[j]
            x1t = x1tp.tile([P, CL * FEAT], bf16)
            x1t_tiles[j] = x1t
            for g in range(CL // 4):
                if xbark and g * 4 >= CL - xbark:
                    nc.scalar.dma_start_transpose(
                        x1t[:, g * 4 * FEAT:(g + 1) * 4 * FEAT].rearrange(
                            "p (c n) -> p c n", c=4),
                        x1n[:, g * 4:(g + 1) * 4, :])
                    continue
                tp = tpsum.tile([P, 4 * FEAT], bf16)
                for kk in range(4):
                    k = g * 4 + kk
                    nc.tensor.transpose(tp[:, kk * FEAT:(kk + 1) * FEAT],
                                        x1n[:, k, :], idn_sb[:])
                nc.scalar.copy(x1t[:, g * 4 * FEAT:(g + 1) * 4 * FEAT], tp[:])

        def emit_mms(j):
            x1t = x1t_tiles[j]
            ob = outp.tile([P, CL * FEAT], odt)
            for h in range(CL // C):
                ps = psum.tile([P, C * FEAT], f32)
                for c in range(C):
                    k = h * C + c
                    t = j * CL + k
                    nc.tensor.matmul(ps[:, c * FEAT:(c + 1) * FEAT],
                                     x2t_all[:, t * P:(t + 1) * P],
                                     wr_sb[:], start=True, stop=False)
                    nc.tensor.matmul(ps[:, c * FEAT:(c + 1) * FEAT],
                                     x1t[:, k * FEAT:(k + 1) * FEAT],
                                     a_bf[:], start=False, stop=True)
                nc.vector.tensor_add(ob[:, h * C * FEAT:(h + 1) * C * FEAT],
                                     ps[:], bias_sb[:])
            if obf32:
                nc.gpsimd.dma_start(outv[j], ob[:])  # SWDGE cast store
            else:
                nc.sync.dma_start(outv[j], ob[:])

        D = min(6, nlc)
        for j in range(D):
            emit_x1_load(j)
            emit_transposes(j)
        # A = M_glob + W_l (placed here on the DVE queue so the first casts
        # aren't head-blocked behind the mg wait)
        nc.vector.tensor_add(a_f32[:], mg_sb[:], wl_sb[:])
        nc.vector.tensor_copy(a_bf[:], a_f32[:])
        for j in range(D, nlc):
            emit_x1_load(j)
            emit_transposes(j)
            emit_mms(j - D)
        for j in range(max(0, nlc - D), nlc):
            emit_mms(j)

    nc.compile()
    return nc


def _get_nc(rows_per_core: int, xbark: int = 0, obf32: bool = False):
    key = (rows_per_core, xbark, obf32)
    if key not in _nc_cache:
        _nc_cache[key] = _build(rows_per_core, xbark, obf32)
    return _nc_cache[key]


def make_in_maps(input_left, input_right, U, W_l, W_r, bias, n_total_rows):
    """Host-side prep: shard rows, lay out the small weights."""
    x1 = np.ascontiguousarray(np.asarray(input_left, np.float32)).reshape(-1, FEAT)
    x2 = np.ascontiguousarray(np.asarray(input_right, np.float32)).reshape(-1, FEAT)
    U = np.asarray(U, np.float32)
    rows = x1.shape[0] // N_CORES
    # up[r, o*128+l] = U[o, l, r]
    up = np.ascontiguousarray(U.transpose(2, 0, 1).reshape(FEAT, FEAT * FEAT)
                              ).astype(ml_dtypes.bfloat16)
    wl = np.ascontiguousarray(np.asarray(W_l, np.float32))
    wr = np.ascontiguousarray(np.asarray(W_r, np.float32)).astype(ml_dtypes.float8_e4m3)
    nb = (np.float64(n_total_rows) * np.asarray(bias, np.float64)).astype(np.float32)
    bias_tiled = np.ascontiguousarray(np.tile(nb, (128, C)))
    ident = np.eye(128, dtype=ml_dtypes.bfloat16)
    in_maps = []
    for c in range(N_CORES):
        in_maps.append({
            "input_left": x1[c * rows:(c + 1) * rows],
            "input_right": x2[c * rows:(c + 1) * rows],
            "u_prep": up,
            "w_l": wl,
            "w_r": wr,
            "bias_tiled": bias_tiled,
            "ident": ident,
        })
    return in_maps, rows


def kernel(input_left, input_right, U, W_l, W_r, bias):
    from concourse.bass_utils import run_bass_kernel_spmd

    lead = np.asarray(input_left).shape[:-1]
    n_total = int(np.prod(lead))
    in_maps, rows = make_in_maps(input_left, input_right, U, W_l, W_r, bias,
                                 n_total)
    nc = _get_nc(rows)
    res = run_bass_kernel_spmd(nc, in_maps, core_ids=list(range(N_CORES)))
    out = np.concatenate(
        [np.asarray(r["out"], np.float32) for r in res.results], axis=0)
    return out.reshape(lead + (FEAT,))


# revision 11
# speedup vs baseline: 1.1095x; 1.1095x over previous
# Bass/Trainium2 kernel for nn_Bilinear_46660524703902.
#
# Math (see reference):
#   s    = sum_n x2[n, :]                        # [R] global row-sum
#   M    = einsum('olr,r->lo', U, s)             # [L, O]
#   out  = x1 @ (M + W_l) + x2 @ W_r + N * bias  # [N, O]
#
# Distribution: data-parallel over the flattened row axis across 8 cores.
# M is linear in s, so each core computes M_c from its local row-sum and one
# 64KB AllReduce of M_c yields the global M.
#
# Dataflow (per core, rows=65536, 32 chunks of 16 128-row tiles):
#   Phase A: SWDGE cast-load x2 chunks fp32->bf16 on the gpsimd queue
#            (8KB/partition descriptors), transpose each 128x128 tile on the
#            PE (is_transpose matmul -> bf16 PSUM), ACT copies PSUM->SBUF
#            into the resident x2T [r, n] (16MB bf16), DVE row-sums.
#   M:       stream U' [r,(o l)] bf16 from DRAM through the ld2 pool in 8
#            pieces; 16 PE matmuls per piece (lhsT=U' slice, rhs=s) -> M_c;
#            AllReduce(M_c) on gpsimd; readback also on gpsimd.
#   Phase B: x1 loads run as plain fp32 HWDGE DMAs on the *sync* queue so
#            they keep streaming while gpsimd sits in the collective's
#            completion wait; DVE casts fp32->bf16 (it has slack), PE
#            transposes the bf16 tiles (tpsum+ACT copy like phase A), then
#            per 128-row tile two accumulating matmuls into PSUM [n, o]:
#              psum  = x2T_tile.T @ W_r
#              psum += x1T_tile.T @ A          (A = M + W_l)
#            DVE adds the pre-tiled N*bias writing bf16, stores bf16 to HBM
#            (4KB/partition descriptors), also on sync (fire-and-forget).
# No xbar DMA transposes and bf16 stores keep per-SDMA-engine work at
# ~260us vs the 2.16ms baseline's ~500us.

import numpy as np
import ml_dtypes
from contextlib import ExitStack

N_CORES = 8
FEAT = 128  # L == R == O == 128
CL = 16     # 128-row tiles per load chunk (2048 rows)
C = 8       # tiles per PSUM accumulation group

_nc_cache: dict = {}


def _build(rows_per_core: int, xbark: int = 0, obf32: bool = False):
    """Build + compile the per-core Bass module (same program on all cores).

    xbark: number of x1 tiles per chunk transposed via xbar DMA instead of
           the PE (0..CL), for PE/DMA load balancing.
    obf32: store fp32 output via SWDGE instead of bf16 via sync (fallback).
    """
    from concourse import bacc, mybir, tile

    f32 = mybir.dt.float32
    bf16 = mybir.dt.bfloat16
    fp8 = mybir.dt.float8e4
    X = mybir.AxisListType.X

    P = 128
    assert rows_per_core % (P * CL) == 0
    nlc = rows_per_core // (P * CL)  # load chunks per tensor (32 full size)
    odt = f32 if obf32 else bf16

    nc = bacc.Bacc("TRN2", target_bir_lowering=False, debug=False,
                   num_devices=N_CORES)

    x1 = nc.dram_tensor("input_left", [rows_per_core, FEAT], f32,
                        kind="ExternalInput")
    x2 = nc.dram_tensor("input_right", [rows_per_core, FEAT], f32,
                        kind="ExternalInput")
    up = nc.dram_tensor("u_prep", [FEAT, FEAT * FEAT], bf16,
                        kind="ExternalInput")  # [r, (o l)] = U[o, l, r]
    wl = nc.dram_tensor("w_l", [FEAT, FEAT], f32, kind="ExternalInput")
    wr = nc.dram_tensor("w_r", [FEAT, FEAT], fp8, kind="ExternalInput")
    biasT = nc.dram_tensor("bias_tiled", [P, C * FEAT], f32,
                           kind="ExternalInput")  # N*bias tiled C times
    idn = nc.dram_tensor("ident", [P, FEAT], bf16, kind="ExternalInput")
    out = nc.dram_tensor("out", [rows_per_core, FEAT], odt,
                         kind="ExternalOutput")

    # Row->partition mapping: within chunk j, partition p holds the 16
    # consecutive rows j*2048 + p*16 + k, so every HBM load runs 8KB
    # contiguous per partition (stores: 4KB bf16). Transposing tile k of a
    # chunk yields [r, p] columns where column p is row p*16+k, so psum
    # partition i of the phase-B matmuls corresponds to row i*16+k,
    # matching the (k o) store layout below.
    x2v = x2.ap().rearrange("(h p k) r -> h p k r", p=P, k=CL)
    x1v = x1.ap().rearrange("(h p k) r -> h p k r", p=P, k=CL)
    outv = out.ap().rearrange("(h p k) o -> h p (k o)", p=P, k=CL)
    upv = up.ap().rearrange("r (g f) -> g r f", g=8)

    with tile.TileContext(nc) as tc, ExitStack() as ctx:
        consts = ctx.enter_context(tc.tile_pool(name="consts", bufs=1))
        big = ctx.enter_context(tc.tile_pool(name="big", bufs=1))
        ld2 = ctx.enter_context(tc.tile_pool(name="ld2", bufs=3))
        ld1f = ctx.enter_context(tc.tile_pool(name="ld1f", bufs=4))
        ld1b = ctx.enter_context(tc.tile_pool(name="ld1b", bufs=6))
        x1tp = ctx.enter_context(tc.tile_pool(name="x1t", bufs=6))
        outp = ctx.enter_context(tc.tile_pool(name="outp", bufs=3))
        psum = ctx.enter_context(tc.tile_pool(name="psum", bufs=2, space="PSUM"))
        tpsum = ctx.enter_context(tc.tile_pool(name="tpsum", bufs=3, space="PSUM"))
        mpsum = ctx.enter_context(tc.tile_pool(name="mpsum", bufs=1, space="PSUM"))
        dram = ctx.enter_context(tc.tile_pool(name="dram", bufs=1, space="DRAM"))

        # Constants, loaded once
        wl_sb = consts.tile([FEAT, FEAT], f32)
        wr_sb = consts.tile([FEAT, FEAT], fp8)
        bias_sb = consts.tile([P, C * FEAT], f32)
        idn_sb = consts.tile([P, FEAT], bf16)
        nc.sync.dma_start(wl_sb[:], wl[:])
        nc.sync.dma_start(wr_sb[:], wr[:])
        nc.sync.dma_start(bias_sb[:], biasT[:])
        nc.sync.dma_start(idn_sb[:], idn[:])

        # Persistent working tiles
        x2t_all = big.tile([P, rows_per_core], fp8)  # [r, n] resident 8MB
        s_cols = consts.tile([P, max(4 * nlc, 1)], f32)
        s_f32 = consts.tile([P, 1], f32)
        s_bf = consts.tile([P, 1], bf16)
        m_sb = consts.tile([FEAT, FEAT], f32)
        mg_sb = consts.tile([FEAT, FEAT], f32)
        a_f32 = consts.tile([FEAT, FEAT], f32)
        a_bf = consts.tile([FEAT, FEAT], bf16)

        # ---- Phase A: stream x2, PE-transpose into residency, row-sums
        for j in range(nlc):
            x2n = ld2.tile([P, CL, FEAT], bf16)
            nc.gpsimd.dma_start(x2n[:], x2v[j])  # fp32 -> bf16 cast
            for g in range(CL // 4):
                tp = tpsum.tile([P, 4 * FEAT], bf16)
                for kk in range(4):
                    k = g * 4 + kk
                    nc.tensor.transpose(tp[:, kk * FEAT:(kk + 1) * FEAT],
                                        x2n[:, k, :], idn_sb[:])
                nc.scalar.copy(
                    x2t_all[:, (j * CL + g * 4) * P:(j * CL + g * 4 + 4) * P],
                    tp[:])
                nc.vector.reduce_sum(s_cols[:, j * 4 + g:j * 4 + g + 1],
                                     tp[:], axis=X)

        # ---- x1 loads: plain fp32 on the sync queue (HWDGE) so nothing
        # blocks them; DVE casts to bf16 into ld1b.
        x1n_tiles = [None] * nlc

        def emit_x1_load(j):
            x1f = ld1f.tile([P, CL, FEAT], f32)
            nc.sync.dma_start(x1f[:], x1v[j])
            x1b = ld1b.tile([P, CL, FEAT], bf16)
            nc.vector.tensor_copy(x1b[:], x1f[:])
            x1n_tiles[j] = x1b

        # ---- M_c = einsum(U, s_local): stream U' through ld2, 128 matmuls
        nc.vector.reduce_sum(s_f32[:], s_cols[:], axis=X)
        nc.vector.tensor_copy(s_bf[:], s_f32[:])
        m_ps = mpsum.tile([FEAT, FEAT], f32)
        for g in range(8):
            upg = ld2.tile([P, 16 * FEAT], bf16)
            nc.sync.dma_start(upg[:], upv[g])
            for oo in range(16):
                o = g * 16 + oo
                nc.tensor.matmul(m_ps[:, o:o + 1],
                                 upg[:, oo * FEAT:(oo + 1) * FEAT],
                                 s_bf[:], start=True, stop=True)
        nc.vector.tensor_copy(m_sb[:], m_ps[:])
        m_loc = dram.tile([FEAT, FEAT], f32)
        m_glob = dram.tile([FEAT, FEAT], f32)
        nc.sync.dma_start(m_loc[:], m_sb[:])

        # gpsimd queue: just the collective + readback; x1 loads and stores
        # never queue behind it.
        nc.gpsimd.collective_compute(
            "AllReduce", mybir.AluOpType.add,
            replica_groups=[list(range(N_CORES))],
            ins=[m_loc.opt()], outs=[m_glob.opt()])
        nc.gpsimd.dma_start(mg_sb[:], m_glob[:])

        # ---- Phase B: PE-transpose x1, matmuls, bias, store. Transposes
        # lead matmul emission by D chunks (= x1t bufs) so the PE queue
        # isn't head-blocked on the a_bf wait during the collective.
        x1t_tiles = [None] * nlc

        def emit_transposes(j):
            x1n = x1n_tiles[j]
            x1t = x1tp.tile([P, CL * FEAT], bf16)
            x1t_tiles[j] = x1t
            for g in range(CL // 4):
                if xbark and g * 4 >= CL - xbark:
                    nc.scalar.dma_start_transpose(
                        x1t[:, g * 4 * FEAT:(g + 1) * 4 * FEAT].rearrange(
                            "p (c n) -> p c n", c=4),
                        x1n[:, g * 4:(g + 1) * 4, :])
                    continue
                tp = tpsum.tile([P, 4 * FEAT], bf16)
                for kk in range(4):
                    k = g * 4 + kk
                    nc.tensor.transpose(tp[:, kk * FEAT:(kk + 1) * FEAT],
                                        x1n[:, k, :], idn_sb[:])
                nc.scalar.copy(x1t[:, g * 4 * FEAT:(g + 1) * 4 * FEAT], tp[:])

        def emit_mms(j):
            x1t = x1t_tiles[j]
            ob = outp.tile([P, CL * FEAT], odt)
            for h in range(CL // C):
                ps = psum.tile([P, C * FEAT], f32)
                for c in range(C):
                    k = h * C + c
                    t = j * CL + k
                    nc.tensor.matmul(ps[:, c * FEAT:(c + 1) * FEAT],
                                     x2t_all[:, t * P:(t + 1) * P],
                                     wr_sb[:], start=True, stop=False)
                    nc.tensor.matmul(ps[:, c * FEAT:(c + 1) * FEAT],
                                     x1t[:, k * FEAT:(k + 1) * FEAT],
                                     a_bf[:], start=False, stop=True)
                nc.vector.tensor_add(ob[:, h * C * FEAT:(h + 1) * C * FEAT],
                                     ps[:], bias_sb[:])
            if obf32:
                nc.gpsimd.dma_start(outv[j], ob[:])  # SWDGE cast store
            else:
                nc.sync.dma_start(outv[j], ob[:])

        D = min(6, nlc)
        for j in range(D):
            emit_x1_load(j)
            emit_transposes(j)
        # A = M_glob + W_l (placed here on the DVE queue so the first casts
        # aren't head-blocked behind the mg wait)
        nc.vector.tensor_add(a_f32[:], mg_sb[:], wl_sb[:])
        nc.vector.tensor_copy(a_bf[:], a_f32[:])
        for j in range(D, nlc):
            emit_x1_load(j)
            emit_transposes(j)
            emit_mms(j - D)
        for j in range(max(0, nlc - D), nlc):
            emit_mms(j)

    nc.compile()
    return nc


def _get_nc(rows_per_core: int, xbark: int = 0, obf32: bool = False):
    key = (rows_per_core, xbark, obf32)
    if key not in _nc_cache:
        _nc_cache[key] = _build(rows_per_core, xbark, obf32)
    return _nc_cache[key]


def make_in_maps(input_left, input_right, U, W_l, W_r, bias, n_total_rows):
    """Host-side prep: shard rows, lay out the small weights."""
    x1 = np.ascontiguousarray(np.asarray(input_left, np.float32)).reshape(-1, FEAT)
    x2 = np.ascontiguousarray(np.asarray(input_right, np.float32)).reshape(-1, FEAT)
    U = np.asarray(U, np.float32)
    rows = x1.shape[0] // N_CORES
    # up[r, o*128+l] = U[o, l, r]
    up = np.ascontiguousarray(U.transpose(2, 0, 1).reshape(FEAT, FEAT * FEAT)
                              ).astype(ml_dtypes.bfloat16)
    wl = np.ascontiguousarray(np.asarray(W_l, np.float32))
    wr = np.ascontiguousarray(np.asarray(W_r, np.float32)).astype(ml_dtypes.float8_e4m3)
    nb = (np.float64(n_total_rows) * np.asarray(bias, np.float64)).astype(np.float32)
    bias_tiled = np.ascontiguousarray(np.tile(nb, (128, C)))
    ident = np.eye(128, dtype=ml_dtypes.bfloat16)
    in_maps = []
    for c in range(N_CORES):
        in_maps.append({
            "input_left": x1[c * rows:(c + 1) * rows],
            "input_right": x2[c * rows:(c + 1) * rows],
            "u_prep": up,
            "w_l": wl,
            "w_r": wr,
            "bias_tiled": bias_tiled,
            "ident": ident,
        })
    return in_maps, rows


def kernel(input_left, input_right, U, W_l, W_r, bias):
    from concourse.bass_utils import run_bass_kernel_spmd

    lead = np.asarray(input_left).shape[:-1]
    n_total = int(np.prod(lead))
    in_maps, rows = make_in_maps(input_left, input_right, U, W_l, W_r, bias,
                                 n_total)
    nc = _get_nc(rows)
    res = run_bass_kernel_spmd(nc, in_maps, core_ids=list(range(N_CORES)))
    out = np.concatenate(
        [np.asarray(r["out"], np.float32) for r in res.results], axis=0)
    return out.reshape(lead + (FEAT,))


# revision 12
# speedup vs baseline: 1.1887x; 1.0713x over previous
# Bass/Trainium2 kernel for nn_Bilinear_46660524703902.
#
# Math (see reference):
#   s    = sum_n x2[n, :]                        # [R] global row-sum
#   M    = einsum('olr,r->lo', U, s)             # [L, O]
#   out  = x1 @ (M + W_l) + x2 @ W_r + N * bias  # [N, O]
#
# Distribution: data-parallel over the flattened row axis across 8 cores.
# M is linear in s, so each core computes M_c from its local row-sum and one
# 64KB AllReduce of M_c yields the global M.
#
# Dataflow (per core, rows=65536, 32 chunks of 16 128-row tiles):
#   Phase A: SWDGE cast-load x2 chunks fp32->bf16 on the gpsimd queue
#            (8KB/partition descriptors), transpose each 128x128 tile on the
#            PE (is_transpose matmul -> bf16 PSUM), ACT copies PSUM->SBUF
#            into the resident x2T [r, n] (16MB bf16), DVE row-sums.
#   M:       stream U' [r,(o l)] bf16 from DRAM through the ld2 pool in 8
#            pieces; 16 PE matmuls per piece (lhsT=U' slice, rhs=s) -> M_c;
#            AllReduce(M_c) on gpsimd; readback also on gpsimd.
#   Phase B: x1 loads run as plain fp32 HWDGE DMAs on the *sync* queue so
#            they keep streaming while gpsimd sits in the collective's
#            completion wait; DVE casts fp32->bf16 (it has slack), PE
#            transposes the bf16 tiles (tpsum+ACT copy like phase A), then
#            per 128-row tile two accumulating matmuls into PSUM [n, o]:
#              psum  = x2T_tile.T @ W_r
#              psum += x1T_tile.T @ A          (A = M + W_l)
#            DVE adds the pre-tiled N*bias writing bf16, stores bf16 to HBM
#            (4KB/partition descriptors), also on sync (fire-and-forget).
# No xbar DMA transposes and bf16 stores keep per-SDMA-engine work at
# ~260us vs the 2.16ms baseline's ~500us.

import numpy as np
import ml_dtypes
from contextlib import ExitStack

N_CORES = 8
FEAT = 128  # L == R == O == 128
CL = 16     # 128-row tiles per load chunk (2048 rows)
C = 8       # tiles per PSUM accumulation group

_nc_cache: dict = {}


def _build(rows_per_core: int, xbark: int = 0, obf32: bool = False):
    """Build + compile the per-core Bass module (same program on all cores).

    xbark: number of x1 tiles per chunk transposed via xbar DMA instead of
           the PE (0..CL), for PE/DMA load balancing.
    obf32: store fp32 output via SWDGE instead of bf16 via sync (fallback).
    """
    from concourse import bacc, mybir, tile

    f32 = mybir.dt.float32
    bf16 = mybir.dt.bfloat16
    fp8 = mybir.dt.float8e4
    X = mybir.AxisListType.X

    P = 128
    assert rows_per_core % (P * CL) == 0
    nlc = rows_per_core // (P * CL)  # load chunks per tensor (32 full size)
    odt = f32 if obf32 else bf16

    nc = bacc.Bacc("TRN2", target_bir_lowering=False, debug=False,
                   num_devices=N_CORES)

    x1 = nc.dram_tensor("input_left", [rows_per_core, FEAT], f32,
                        kind="ExternalInput")
    x2 = nc.dram_tensor("input_right", [rows_per_core, FEAT], f32,
                        kind="ExternalInput")
    up = nc.dram_tensor("u_prep", [FEAT, FEAT * FEAT], bf16,
                        kind="ExternalInput")  # [r, (o l)] = U[o, l, r]
    wl = nc.dram_tensor("w_l", [FEAT, FEAT], f32, kind="ExternalInput")
    wr = nc.dram_tensor("w_r", [FEAT, FEAT], fp8, kind="ExternalInput")
    biasT = nc.dram_tensor("bias_tiled", [P, C * FEAT], f32,
                           kind="ExternalInput")  # N*bias tiled C times
    idn = nc.dram_tensor("ident", [P, FEAT], bf16, kind="ExternalInput")
    out = nc.dram_tensor("out", [rows_per_core, FEAT], odt,
                         kind="ExternalOutput")

    # Row->partition mapping: within chunk j, partition p holds the 16
    # consecutive rows j*2048 + p*16 + k, so every HBM load runs 8KB
    # contiguous per partition (stores: 4KB bf16). Transposing tile k of a
    # chunk yields [r, p] columns where column p is row p*16+k, so psum
    # partition i of the phase-B matmuls corresponds to row i*16+k,
    # matching the (k o) store layout below.
    x2v = x2.ap().rearrange("(h p k) r -> h p k r", p=P, k=CL)
    x1v = x1.ap().rearrange("(h p k) r -> h p k r", p=P, k=CL)
    outv = out.ap().rearrange("(h p k) o -> h p (k o)", p=P, k=CL)
    upv = up.ap().rearrange("r (g f) -> g r f", g=8)

    with tile.TileContext(nc) as tc, ExitStack() as ctx:
        consts = ctx.enter_context(tc.tile_pool(name="consts", bufs=1))
        big = ctx.enter_context(tc.tile_pool(name="big", bufs=1))
        ld2 = ctx.enter_context(tc.tile_pool(name="ld2", bufs=3))
        ld1f = ctx.enter_context(tc.tile_pool(name="ld1f", bufs=4))
        ld1b = ctx.enter_context(tc.tile_pool(name="ld1b", bufs=6))
        x1tp = ctx.enter_context(tc.tile_pool(name="x1t", bufs=6))
        outp = ctx.enter_context(tc.tile_pool(name="outp", bufs=3))
        psum = ctx.enter_context(tc.tile_pool(name="psum", bufs=2, space="PSUM"))
        tpsum = ctx.enter_context(tc.tile_pool(name="tpsum", bufs=3, space="PSUM"))
        mpsum = ctx.enter_context(tc.tile_pool(name="mpsum", bufs=1, space="PSUM"))
        dram = ctx.enter_context(tc.tile_pool(name="dram", bufs=1, space="DRAM"))

        # Constants, loaded once
        wl_sb = consts.tile([FEAT, FEAT], f32)
        wr_sb = consts.tile([FEAT, FEAT], fp8)
        bias_sb = consts.tile([P, C * FEAT], f32)
        idn_sb = consts.tile([P, FEAT], bf16)
        nc.sync.dma_start(wl_sb[:], wl[:])
        nc.sync.dma_start(wr_sb[:], wr[:])
        nc.sync.dma_start(bias_sb[:], biasT[:])
        nc.sync.dma_start(idn_sb[:], idn[:])

        # Persistent working tiles
        x2t_all = big.tile([P, rows_per_core], fp8)  # [r, n] resident 8MB
        s_cols = consts.tile([P, max(4 * nlc, 1)], f32)
        s_f32 = consts.tile([P, 1], f32)
        s_bf = consts.tile([P, 1], bf16)
        m_sb = consts.tile([FEAT, FEAT], f32)
        mg_sb = consts.tile([FEAT, FEAT], f32)
        a_f32 = consts.tile([FEAT, FEAT], f32)
        a_bf = consts.tile([FEAT, FEAT], bf16)

        # ---- Phase A: stream x2, PE-transpose into residency, row-sums
        for j in range(nlc):
            x2n = ld2.tile([P, CL, FEAT], bf16)
            nc.gpsimd.dma_start(x2n[:], x2v[j])  # fp32 -> bf16 cast
            for g in range(CL // 4):
                tp = tpsum.tile([P, 4 * FEAT], bf16)
                for kk in range(4):
                    k = g * 4 + kk
                    nc.tensor.transpose(tp[:, kk * FEAT:(kk + 1) * FEAT],
                                        x2n[:, k, :], idn_sb[:])
                nc.scalar.copy(
                    x2t_all[:, (j * CL + g * 4) * P:(j * CL + g * 4 + 4) * P],
                    tp[:])
                nc.vector.reduce_sum(s_cols[:, j * 4 + g:j * 4 + g + 1],
                                     tp[:], axis=X)

        # ---- x1 loads: plain fp32 on the sync queue (HWDGE) so nothing
        # blocks them; DVE casts to bf16 into ld1b.
        x1n_tiles = [None] * nlc

        def emit_x1_load(j):
            x1f = ld1f.tile([P, CL, FEAT], f32)
            nc.sync.dma_start(x1f[:], x1v[j])
            x1b = ld1b.tile([P, CL, FEAT], bf16)
            nc.vector.tensor_copy(x1b[:], x1f[:])
            x1n_tiles[j] = x1b

        # ---- M_c = einsum(U, s_local): stream U' through ld2, 128 matmuls
        nc.vector.reduce_sum(s_f32[:], s_cols[:], axis=X)
        nc.vector.tensor_copy(s_bf[:], s_f32[:])
        m_ps = mpsum.tile([FEAT, FEAT], f32)
        for g in range(8):
            upg = ld2.tile([P, 16 * FEAT], bf16)
            nc.sync.dma_start(upg[:], upv[g])
            for oo in range(16):
                o = g * 16 + oo
                nc.tensor.matmul(m_ps[:, o:o + 1],
                                 upg[:, oo * FEAT:(oo + 1) * FEAT],
                                 s_bf[:], start=True, stop=True)
        nc.vector.tensor_copy(m_sb[:], m_ps[:])
        m_loc = dram.tile([FEAT, FEAT], f32)
        m_glob = dram.tile([FEAT, FEAT], f32)
        nc.sync.dma_start(m_loc[:], m_sb[:])

        # gpsimd queue: just the collective + readback; x1 loads and stores
        # never queue behind it.
        nc.gpsimd.collective_compute(
            "AllReduce", mybir.AluOpType.add,
            replica_groups=[list(range(N_CORES))],
            ins=[m_loc.opt()], outs=[m_glob.opt()])
        nc.gpsimd.dma_start(mg_sb[:], m_glob[:])

        # ---- Phase B: PE-transpose x1, matmuls, bias, store. Transposes
        # lead matmul emission by D chunks (= x1t bufs) so the PE queue
        # isn't head-blocked on the a_bf wait during the collective.
        x1t_tiles = [None] * nlc

        def emit_transposes(j):
            x1n = x1n_tiles[j]
            x1t = x1tp.tile([P, CL * FEAT], bf16)
            x1t_tiles[j] = x1t
            for g in range(CL // 4):
                if xbark and g * 4 >= CL - xbark:
                    nc.scalar.dma_start_transpose(
                        x1t[:, g * 4 * FEAT:(g + 1) * 4 * FEAT].rearrange(
                            "p (c n) -> p c n", c=4),
                        x1n[:, g * 4:(g + 1) * 4, :])
                    continue
                tp = tpsum.tile([P, 4 * FEAT], bf16)
                for kk in range(4):
                    k = g * 4 + kk
                    nc.tensor.transpose(tp[:, kk * FEAT:(kk + 1) * FEAT],
                                        x1n[:, k, :], idn_sb[:])
                nc.scalar.copy(x1t[:, g * 4 * FEAT:(g + 1) * 4 * FEAT], tp[:])

        def emit_mms(j):
            x1t = x1t_tiles[j]
            ob = outp.tile([P, CL * FEAT], odt)
            for h in range(CL // C):
                ps = psum.tile([P, C * FEAT], f32)
                for c in range(C):
                    k = h * C + c
                    t = j * CL + k
                    nc.tensor.matmul(ps[:, c * FEAT:(c + 1) * FEAT],
                                     x1t[:, k * FEAT:(k + 1) * FEAT],
                                     a_bf[:], start=True, stop=False)
                    nc.tensor.matmul(ps[:, c * FEAT:(c + 1) * FEAT],
                                     x2t_all[:, t * P:(t + 1) * P],
                                     wr_sb[:], start=False, stop=True)
                nc.vector.tensor_add(ob[:, h * C * FEAT:(h + 1) * C * FEAT],
                                     ps[:], bias_sb[:])
            if obf32:
                nc.gpsimd.dma_start(outv[j], ob[:])  # SWDGE cast store
            else:
                nc.sync.dma_start(outv[j], ob[:])

        D = min(6, nlc)
        for j in range(D):
            emit_x1_load(j)
            emit_transposes(j)
        # A = M_glob + W_l (placed here on the DVE queue so the first casts
        # aren't head-blocked behind the mg wait)
        nc.vector.tensor_add(a_f32[:], mg_sb[:], wl_sb[:])
        nc.vector.tensor_copy(a_bf[:], a_f32[:])
        for j in range(D, nlc):
            emit_x1_load(j)
            emit_transposes(j)
            emit_mms(j - D)
        for j in range(max(0, nlc - D), nlc):
            emit_mms(j)

    nc.compile()
    return nc


def _get_nc(rows_per_core: int, xbark: int = 0, obf32: bool = False):
    key = (rows_per_core, xbark, obf32)
    if key not in _nc_cache:
        _nc_cache[key] = _build(rows_per_core, xbark, obf32)
    return _nc_cache[key]


def make_in_maps(input_left, input_right, U, W_l, W_r, bias, n_total_rows):
    """Host-side prep: shard rows, lay out the small weights."""
    x1 = np.ascontiguousarray(np.asarray(input_left, np.float32)).reshape(-1, FEAT)
    x2 = np.ascontiguousarray(np.asarray(input_right, np.float32)).reshape(-1, FEAT)
    U = np.asarray(U, np.float32)
    rows = x1.shape[0] // N_CORES
    # up[r, o*128+l] = U[o, l, r]
    up = np.ascontiguousarray(U.transpose(2, 0, 1).reshape(FEAT, FEAT * FEAT)
                              ).astype(ml_dtypes.bfloat16)
    wl = np.ascontiguousarray(np.asarray(W_l, np.float32))
    wr = np.ascontiguousarray(np.asarray(W_r, np.float32)).astype(ml_dtypes.float8_e4m3)
    nb = (np.float64(n_total_rows) * np.asarray(bias, np.float64)).astype(np.float32)
    bias_tiled = np.ascontiguousarray(np.tile(nb, (128, C)))
    ident = np.eye(128, dtype=ml_dtypes.bfloat16)
    in_maps = []
    for c in range(N_CORES):
        in_maps.append({
            "input_left": x1[c * rows:(c + 1) * rows],
            "input_right": x2[c * rows:(c + 1) * rows],
            "u_prep": up,
            "w_l": wl,
            "w_r": wr,
            "bias_tiled": bias_tiled,
            "ident": ident,
        })
    return in_maps, rows


def kernel(input_left, input_right, U, W_l, W_r, bias):
    from concourse.bass_utils import run_bass_kernel_spmd

    lead = np.asarray(input_left).shape[:-1]
    n_total = int(np.prod(lead))
    in_maps, rows = make_in_maps(input_left, input_right, U, W_l, W_r, bias,
                                 n_total)
    nc = _get_nc(rows)
    res = run_bass_kernel_spmd(nc, in_maps, core_ids=list(range(N_CORES)))
    out = np.concatenate(
        [np.asarray(r["out"], np.float32) for r in res.results], axis=0)
    return out.reshape(lead + (FEAT,))
